# revision 1
# baseline (speedup 1.0000x reference)
"""Trainium2 Bass kernel for nn_KernelProjectionT2I.

Sharding: data-parallel over the caption axis (B_cap=48 -> 6 captions per
core on 8 cores). Each core holds the full image batch + conv weights and
computes the (B_img, 6) similarity columns for its captions; the host
concatenates the per-core columns.

Math (per caption q):
  cap0 = cap_embed[q, 0]                          (1024,)
  cap_repr = Wred @ cap0 + bred                   (256,)
  wdyn = softmax_K((Wproj @ cap_repr + bproj).reshape(1024, 3))
  Xconv[c, n] = w0[c] x[c, r-1] + w1[c] x[c, r] + w2[c] x[c, r+1]
  y = Wconv @ Xconv            (bias bconv dropped: softmax over regions is
                                shift-invariant, so pooled = pooled' + bconv)
  A = sum_r exp(y), Bsum = sum_r y exp(y)         (gated pool, per (b, d))
  img_vec = Bsum/A + bconv ;  sims[b, q] = <img_vec, cap0> / (|img_vec||cap0|)

Device layout: n = (b, r) on PSUM partitions for y (so region sums run on the
TensorEngine as 0/1-selector matmuls), channel c on SBUF partitions for the
depthwise stage (per-partition-scalar fused MACs). Big matmuls use float32r
(FP22 single pass).
"""

import numpy as np
from contextlib import ExitStack

import concourse.bass as bass
import concourse.tile as tile
from concourse import bacc, mybir
from concourse.bass_utils import run_bass_kernel_spmd

F32 = mybir.dt.float32
F32R = mybir.dt.float32r
AF = mybir.ActivationFunctionType
OP = mybir.AluOpType

N_CORES = 8
B, R, D = 48, 36, 1024
Q = 48
QL = Q // N_CORES          # 6 captions per core
DQ, DK, K = 256, 3072, 3
NB = B * R                 # 1728
NP = 1792                  # padded to 14 chunks of 128
NCH = NP // 128            # 14
# Xconv slabs, both b-aligned and 128-aligned (lcm(36,128)=1152)
SLABS = [(0, 32, 0, 9), (32, 16, 9, 5)]   # (b0, nb, nch0, n_nchunks)

LAST_EXEC_NS = None
_CACHE = {}
import os
STAGE = int(os.environ.get("KSTAGE", "9"))


def _build_nc():
    nc = bacc.Bacc(trn_type="TRN2", target_bir_lowering=False,
                   num_devices=N_CORES)
    x38_d = nc.dram_tensor("x38", [128, 8, B, 38], F32, kind="ExternalInput")
    wct_d = nc.dram_tensor("wct", [128, 8, D], F32R, kind="ExternalInput")
    wrt_d = nc.dram_tensor("wrt", [128, 8, DQ], F32, kind="ExternalInput")
    wpp_d = nc.dram_tensor("wpp", [128, 2, K, D], F32, kind="ExternalInput")
    bred_d = nc.dram_tensor("bred", [128, 2], F32, kind="ExternalInput")
    bpp_d = nc.dram_tensor("bpp", [128, 8, K], F32, kind="ExternalInput")
    sel_d = nc.dram_tensor("sel", [128, NCH, B], F32R, kind="ExternalInput")
    bcb_d = nc.dram_tensor("bcb", [B, D], F32, kind="ExternalInput")
    capt_d = nc.dram_tensor("capt", [128, 8, QL], F32, kind="ExternalInput")
    capb_d = nc.dram_tensor("capb", [QL, B, D], F32, kind="ExternalInput")
    out_d = nc.dram_tensor("out", [B, QL], F32, kind="ExternalOutput")

    with ExitStack() as ctx:
        tc = ctx.enter_context(tile.TileContext(nc))
        const = ctx.enter_context(tc.tile_pool(name="const", bufs=1))
        xcp = ctx.enter_context(tc.tile_pool(name="xcp", bufs=2))
        ep = ctx.enter_context(tc.tile_pool(name="ep", bufs=2))
        qv = ctx.enter_context(tc.tile_pool(name="qv", bufs=1))
        small = ctx.enter_context(tc.tile_pool(name="small", bufs=2))
        psy = ctx.enter_context(tc.tile_pool(name="psy", bufs=2, space="PSUM"))
        psA = ctx.enter_context(tc.tile_pool(name="psA", bufs=1, space="PSUM"))
        psB = ctx.enter_context(tc.tile_pool(name="psB", bufs=1, space="PSUM"))

        # ---- resident inputs ----
        capt_t = const.tile([128, 8, QL], F32)
        nc.sync.dma_start(out=capt_t, in_=capt_d.ap())
        bred_t = const.tile([128, 2], F32)
        nc.sync.dma_start(out=bred_t, in_=bred_d.ap())
        bpp_t = const.tile([128, 8, K], F32)
        nc.sync.dma_start(out=bpp_t, in_=bpp_d.ap())
        x38_t = const.tile([128, 8, B, 38], F32)
        nc.sync.dma_start(out=x38_t, in_=x38_d.ap())
        wct_t = const.tile([128, 8, D], F32R)
        nc.sync.dma_start(out=wct_t, in_=wct_d.ap())
        sel_t = const.tile([128, NCH, B], F32R)
        nc.sync.dma_start(out=sel_t, in_=sel_d.ap())
        bcb_t = const.tile([B, D], F32)
        nc.sync.dma_start(out=bcb_t, in_=bcb_d.ap())

        # MLP weights share the Xconv slab slots (used once, up front)
        wrt_t = xcp.tile([128, 8, DQ], F32, tag="xc")
        nc.sync.dma_start(out=wrt_t, in_=wrt_d.ap())
        wpp_t = xcp.tile([128, 2, K, D], F32, tag="xc")
        nc.sync.dma_start(out=wpp_t, in_=wpp_d.ap())

        out_sb = const.tile([B, QL], F32)
        nc.vector.memset(out_sb, 0.0)
        z64 = const.tile([128, 64], F32)
        nc.vector.memset(z64, 0.0)
        dot_t = const.tile([B, QL], F32)
        s2_t = const.tile([B, QL], F32)
        s2c_t = const.tile([B, QL], F32)

        # ---- caption MLP for all local captions (full fp32) ----
        repr_ps = psB.tile([128, 2, QL], F32, tag="B")
        for mc in range(2):
            for cc in range(8):
                nc.tensor.matmul(repr_ps[:, mc, :],
                                 lhsT=wrt_t[:, cc, mc * 128:(mc + 1) * 128],
                                 rhs=capt_t[:, cc, :],
                                 start=(cc == 0), stop=(cc == 7))
        repr_sb = small.tile([128, 2, QL], F32)
        for mc in range(2):
            nc.vector.tensor_scalar_add(repr_sb[:, mc, :], repr_ps[:, mc, :],
                                        bred_t[:, mc:mc + 1])

        L_ps = [psy.tile([128, 8, QL], F32, tag="y", name="L0"),
                psy.tile([128, 8, QL], F32, tag="y", name="L1"),
                psA.tile([128, 8, QL], F32, tag="A", name="L2")]
        for kk in range(K):
            for mc in range(8):
                nc.tensor.matmul(L_ps[kk][:, mc, :],
                                 lhsT=wpp_t[:, 0, kk, mc * 128:(mc + 1) * 128],
                                 rhs=repr_sb[:, 0, :], start=True, stop=False)
                nc.tensor.matmul(L_ps[kk][:, mc, :],
                                 lhsT=wpp_t[:, 1, kk, mc * 128:(mc + 1) * 128],
                                 rhs=repr_sb[:, 1, :], start=False, stop=True)

        # softmax over the K taps (no max-sub: |logits| ~ N(0,1))
        e_k = [small.tile([128, 8, QL], F32, name=f"ek{i}") for i in range(K)]
        for kk in range(K):
            for mc in range(8):
                nc.scalar.activation(e_k[kk][:, mc, :], L_ps[kk][:, mc, :],
                                     AF.Exp, bias=bpp_t[:, mc, kk:kk + 1])
        ssum = small.tile([128, 8, QL], F32)
        nc.vector.tensor_add(ssum, e_k[0], e_k[1])
        nc.vector.tensor_add(ssum, ssum, e_k[2])
        rinv = small.tile([128, 8, QL], F32)
        nc.vector.reciprocal(rinv, ssum)
        w_t = [const.tile([128, 8, QL], F32, name=f"w{i}") for i in range(K)]
        for kk in range(K):
            nc.vector.tensor_mul(w_t[kk], e_k[kk], rinv)

        # ---- main loop over local captions ----
        for q in range(QL):
            capb_t = qv.tile([B, D], F32, tag="capb")
            nc.sync.dma_start(out=capb_t, in_=capb_d.ap()[q])

            A_ps = psA.tile([B, D], F32, tag="A")
            B_ps = psB.tile([B, D], F32, tag="B")

            for (b0, nb, nch0, nnch) in SLABS:
                xcv = xcp.tile([128, 8, 1152], F32R, tag="xc")
                cols = nb * 36
                for cc in range(8):
                    xo = xcv[:, cc, 0:cols].rearrange("p (b r) -> p b r", r=36)
                    # xcv = x[r+1]*w2 (ScalarE); then two fused MACs (DVE)
                    nc.scalar.mul(xo, x38_t[:, cc, b0:b0 + nb, 2:38],
                                  w_t[2][:, cc, q:q + 1])
                    nc.vector.scalar_tensor_tensor(
                        xo, x38_t[:, cc, b0:b0 + nb, 0:36],
                        w_t[0][:, cc, q:q + 1], xo, OP.mult, OP.add)
                    nc.vector.scalar_tensor_tensor(
                        xo, x38_t[:, cc, b0:b0 + nb, 1:37],
                        w_t[1][:, cc, q:q + 1], xo, OP.mult, OP.add)
                    if nch0 + nnch == NCH:
                        nc.vector.tensor_copy(
                            out=xcv[:, cc, cols:cols + 64], in_=z64)

                for j in range(nnch):
                    nch = nch0 + j
                    y_ps = psy.tile([128, D], F32, tag="y")
                    for h in range(2):
                        for cc in range(8):
                            nc.tensor.matmul(
                                y_ps[:, h * 512:(h + 1) * 512],
                                lhsT=xcv[:, cc, j * 128:(j + 1) * 128],
                                rhs=wct_t[:, cc, h * 512:(h + 1) * 512],
                                start=(cc == 0), stop=(cc == 7))
                    e_t = ep.tile([128, D], F32R, tag="e")
                    for h in range(2):
                        nc.scalar.activation(e_t[:, h * 512:(h + 1) * 512],
                                             y_ps[:, h * 512:(h + 1) * 512],
                                             AF.Exp)
                    p_t = ep.tile([128, D], F32R, tag="p", bufs=1)
                    nc.vector.tensor_mul(p_t, e_t, y_ps)
                    selr = sel_t[:, nch, :]
                    for h in range(2):
                        nc.tensor.matmul(
                            A_ps[:, h * 512:(h + 1) * 512], lhsT=selr,
                            rhs=e_t[:, h * 512:(h + 1) * 512],
                            start=(nch == 0), stop=(nch == NCH - 1))
                        nc.tensor.matmul(
                            B_ps[:, h * 512:(h + 1) * 512], lhsT=selr,
                            rhs=p_t[:, h * 512:(h + 1) * 512],
                            start=(nch == 0), stop=(nch == NCH - 1))

            # epilogue: img_vec = B/A + bconv, cosine vs caption
            A_sb = qv.tile([B, D], F32, tag="asb")
            nc.scalar.copy(A_sb, A_ps)
            rA = qv.tile([B, D], F32, tag="ra")
            scr = qv.tile([B, D], F32, tag="scr")
            # 1/A via exp(-ln(A)) on ScalarE (A > 0); custom-DVE recip
            # is unsupported on this runtime
            nc.scalar.activation(rA, A_sb, AF.Ln)
            nc.scalar.activation(rA, rA, AF.Exp, scale=-1.0)
            nc.vector.tensor_mul(A_sb, bcb_t, A_sb)   # bconv * A
            nc.vector.tensor_add(A_sb, A_sb, B_ps)    # + B
            v_t = A_sb
            nc.vector.tensor_mul(v_t, v_t, rA)        # img_vec
            nc.vector.tensor_mul(scr, v_t, capb_t)
            nc.vector.tensor_reduce(dot_t[:, q:q + 1], scr,
                                    mybir.AxisListType.X, OP.add)
            nc.vector.tensor_mul(scr, v_t, v_t)
            nc.vector.tensor_reduce(s2_t[:, q:q + 1], scr,
                                    mybir.AxisListType.X, OP.add)
            nc.vector.tensor_mul(scr, capb_t, capb_t)
            nc.vector.tensor_reduce(s2c_t[:, q:q + 1], scr,
                                    mybir.AxisListType.X, OP.add)

        # sims = dot / sqrt(s2 * s2c)  via exp(-0.5 ln(.))
        den = small.tile([B, QL], F32)
        nc.vector.tensor_mul(den, s2_t, s2c_t)
        lg = small.tile([B, QL], F32)
        nc.scalar.activation(lg, den, AF.Ln)
        rs = small.tile([B, QL], F32)
        nc.scalar.activation(rs, lg, AF.Exp, scale=-0.5)
        nc.vector.tensor_mul(out_sb, dot_t, rs)
        nc.sync.dma_start(out=out_d.ap(), in_=out_sb)

    nc.compile()
    return nc


def _chunked(a):
    """(D, ...) -> (128, 8, ...) with d = c*128 + p."""
    return np.ascontiguousarray(
        a.reshape(8, 128, *a.shape[1:]).transpose(1, 0, *range(2, a.ndim + 1)))


def _prep_shared(img, Wred, Wproj, Wconv):
    xt = np.ascontiguousarray(img.transpose(2, 0, 1))       # (D, B, R)
    x38 = np.zeros((D, B, 38), np.float32)
    x38[:, :, 1:37] = xt
    x38 = _chunked(x38)                                      # (128,8,B,38)
    wct = _chunked(np.ascontiguousarray(Wconv.T))            # (128,8,D)
    wrt = _chunked(np.ascontiguousarray(Wred.T))             # (128,8,DQ)
    wpp = np.ascontiguousarray(                              # (128,2,K,D)
        Wproj.reshape(D, K, DQ).transpose(2, 1, 0)
        .reshape(2, 128, K, D).transpose(1, 0, 2, 3))
    sel = np.zeros((128, NCH, B), np.float32)
    n = np.arange(NP)
    valid = n < NB
    sel[n[valid] % 128, n[valid] // 128, n[valid] // R] = 1.0
    return x38, wct, wrt, wpp, sel


def kernel(img_embed, cap_embed, lens, Wred, bred, Wproj, bproj, Wconv,
           bconv, **_unused):
    global LAST_EXEC_NS
    img_embed = np.asarray(img_embed, np.float32)
    cap0 = np.asarray(cap_embed, np.float32)[:, 0, :]        # (Q, D)
    Wred = np.asarray(Wred, np.float32)
    bred_a = np.asarray(bred, np.float32)
    Wproj = np.asarray(Wproj, np.float32)
    bproj_a = np.asarray(bproj, np.float32)
    Wconv = np.asarray(Wconv, np.float32)
    bconv_a = np.asarray(bconv, np.float32)

    if "nc" not in _CACHE:
        _CACHE["nc"] = _build_nc()
    nc = _CACHE["nc"]

    x38, wct, wrt, wpp, sel = _prep_shared(img_embed, Wred, Wproj, Wconv)
    bred_s = np.ascontiguousarray(bred_a.reshape(2, 128).T)
    bpp = _chunked(bproj_a.reshape(D, K))                     # (128,8,K)
    bcb = np.ascontiguousarray(np.broadcast_to(bconv_a, (B, D)))

    in_maps = []
    for c in range(N_CORES):
        capq = cap0[c * QL:(c + 1) * QL]                      # (QL, D)
        capt = _chunked(np.ascontiguousarray(capq.T))         # (128,8,QL)
        capb = np.ascontiguousarray(
            np.broadcast_to(capq[:, None, :], (QL, B, D)))
        in_maps.append({
            "x38": x38, "wct": wct, "wrt": wrt, "wpp": wpp,
            "bred": bred_s, "bpp": bpp, "sel": sel, "bcb": bcb,
            "capt": capt, "capb": capb,
        })

    trace = bool(int(os.environ.get("KTRACE", "0")))
    tdir = os.environ.get("KTRACE_DIR") or None
    res = run_bass_kernel_spmd(nc, in_maps, core_ids=list(range(N_CORES)),
                               trace=trace, tmpdir=tdir)
    LAST_EXEC_NS = res.exec_time_ns
    return np.concatenate([res.results[c]["out"] for c in range(N_CORES)],
                          axis=1)



# revision 10
# speedup vs baseline: 1.0300x; 1.0300x over previous
"""Trainium2 Bass kernel for nn_KernelProjectionT2I.

Sharding: data-parallel over the caption axis (B_cap=48 -> 6 captions per
core on 8 cores). Each core holds the full image batch + conv weights and
computes the (B_img, 6) similarity columns for its captions; the host
concatenates the per-core columns.

Math (per caption q):
  cap0 = cap_embed[q, 0]                          (1024,)
  cap_repr = Wred @ cap0 + bred                   (256,)
  wdyn = softmax_K((Wproj @ cap_repr + bproj).reshape(1024, 3))
  conv[c, n] = w0[c] x[c, r-1] + w1[c] x[c, r] + w2[c] x[c, r+1]
  y = Wconv @ conv               (bconv folded out: softmax over regions is
                                  shift-invariant; pooled = B/A + bconv)
  A = sum_r exp(y), B = sum_r y exp(y)            (gated pool, per (b, d))
  u = B/A ; sims[b, q] = <u + bconv, c_hat> / |u + bconv|   (c_hat host-normed)

Layout: n = (b, r) on PSUM partitions for y; region sums are TensorEngine
0/1-selector matmuls. Contraction c is split 6/8 chunks fp8e4 + DoubleRow
(2x PE throughput; Wconv pre-scaled x32 for fp8 range) and 2/8 chunks
bf16 (caps the fp8 quantization noise: rel err ~1.5e-2 vs the 2e-2 gate).
Depthwise build in bf16 via DVE tensor_scalar (4x) + tensor_tensor (2x);
GpSimd fuses the last tap with the bf16->fp8 cast. e/p and the selector
matmuls stay bf16. ScalarE uses only {Exp, Ln, Prelu} so one activation
table set serves the whole kernel (Prelu alpha=1 == identity for the
per-channel tap-1 multiply).
"""

import os
import numpy as np
from contextlib import ExitStack

import ml_dtypes

import concourse.bass as bass
import concourse.tile as tile
from concourse import bacc, mybir
from concourse.bass_utils import run_bass_kernel_spmd

F32 = mybir.dt.float32
BF16 = mybir.dt.bfloat16
F8 = mybir.dt.float8e4
AF = mybir.ActivationFunctionType
OP = mybir.AluOpType
PM = mybir.MatmulPerfMode

N_CORES = 8
B, R, D = 48, 36, 1024
Q = 48
QL = Q // N_CORES          # 6 captions per core
DQ, K = 256, 3
NB = B * R                 # 1728
NCH = 14                   # ceil(1728/128); chunk 13 has 64 cols
NC8 = 4                    # c-chunks 0..3 via fp8 DoubleRow
NCB = 4                    # c-chunks 4..7 via bf16

WSC = 32.0                 # Wconv pre-scale (fp8 range); exp un-scales
PSC = 4.0                  # p = (y/PSC) e^y keeps p in fp8/bf16 range

LAST_EXEC_NS = None
_CACHE = {}

BF = ml_dtypes.bfloat16
F8NP = mybir.dt.np(F8)


def _build_nc():
    nc = bacc.Bacc(trn_type="TRN2", target_bir_lowering=False,
                   num_devices=N_CORES)
    # MLP inputs (loaded first; gate the caption MLP)
    capt_d = nc.dram_tensor("capt", [128, 8, QL], BF16, kind="ExternalInput")
    wrt_d = nc.dram_tensor("wrt", [128, 8, DQ], BF16, kind="ExternalInput")
    wpp_d = nc.dram_tensor("wpp", [128, 2, K, D], BF16, kind="ExternalInput")
    bred_d = nc.dram_tensor("bred", [128, 2], F32, kind="ExternalInput")
    bpp_d = nc.dram_tensor("bpp", [128, 8, K], F32, kind="ExternalInput")
    # main loop inputs
    x38_d = nc.dram_tensor("x38", [128, 8, B, 38], BF16, kind="ExternalInput")
    wc8_d = nc.dram_tensor("wc8", [128, NC8, D], F8, kind="ExternalInput")
    wcb_d = nc.dram_tensor("wcb", [128, NCB, D], BF16, kind="ExternalInput")
    sel_d = nc.dram_tensor("sel", [128, NCH, B], BF16, kind="ExternalInput")
    bcb_d = nc.dram_tensor("bcb", [B, D], F32, kind="ExternalInput")
    capb_d = nc.dram_tensor("capb", [QL, B, D], BF16, kind="ExternalInput")
    cst_d = nc.dram_tensor("cst", [B, QL + 1], F32, kind="ExternalInput")
    out_d = nc.dram_tensor("out", [B, QL], F32, kind="ExternalOutput")

    with ExitStack() as ctx:
        tc = ctx.enter_context(tile.TileContext(nc))
        const = ctx.enter_context(tc.tile_pool(name="const", bufs=1))
        bld = ctx.enter_context(tc.tile_pool(name="bld", bufs=2))
        xcp = ctx.enter_context(tc.tile_pool(name="xcp", bufs=2))
        ep = ctx.enter_context(tc.tile_pool(name="ep", bufs=2))
        qv = ctx.enter_context(tc.tile_pool(name="qv", bufs=1))
        cbp = ctx.enter_context(tc.tile_pool(name="cbp", bufs=2))
        small = ctx.enter_context(tc.tile_pool(name="small", bufs=2))
        psy = ctx.enter_context(tc.tile_pool(name="psy", bufs=2, space="PSUM"))
        psA = ctx.enter_context(tc.tile_pool(name="psA", bufs=1, space="PSUM"))
        psB = ctx.enter_context(tc.tile_pool(name="psB", bufs=1, space="PSUM"))

        # ---- resident inputs (MLP deps first: DMA order follows issue order)
        capt_t = const.tile([128, 8, QL], BF16)
        nc.sync.dma_start(out=capt_t, in_=capt_d.ap())
        wrt_t = const.tile([128, 8, DQ], BF16)
        nc.sync.dma_start(out=wrt_t, in_=wrt_d.ap())
        wpp_t = const.tile([128, 2, K, D], BF16)
        nc.sync.dma_start(out=wpp_t, in_=wpp_d.ap())
        bred_t = const.tile([128, 2], F32)
        nc.sync.dma_start(out=bred_t, in_=bred_d.ap())
        bpp_t = const.tile([128, 8, K], F32)
        nc.sync.dma_start(out=bpp_t, in_=bpp_d.ap())
        x38_t = const.tile([128, 8, B, 38], BF16)
        nc.sync.dma_start(out=x38_t, in_=x38_d.ap())
        wc8_t = const.tile([128, NC8, D], F8)
        nc.sync.dma_start(out=wc8_t, in_=wc8_d.ap())
        wcb_t = const.tile([128, NCB, D], BF16)
        nc.sync.dma_start(out=wcb_t, in_=wcb_d.ap())
        sel_t = const.tile([128, NCH, B], BF16)
        nc.sync.dma_start(out=sel_t, in_=sel_d.ap())
        bcb_t = const.tile([B, D], F32)
        nc.sync.dma_start(out=bcb_t, in_=bcb_d.ap())
        cst_t = const.tile([B, QL + 1], F32)
        nc.sync.dma_start(out=cst_t, in_=cst_d.ap())

        xc8 = [xcp.tile([128, NC8, NB], F8, name=f"xc8_{i}", tag="x8")
               for i in range(2)]
        xcb = [xcp.tile([128, NCB, NB], BF16, name=f"xcb_{i}", tag="xb")
               for i in range(2)]

        dot_t = const.tile([B, QL], F32)
        s2u_t = const.tile([B, QL], F32)
        s2ub_t = const.tile([B, QL], F32)

        # ---- caption MLP for all local captions ----
        repr_ps = psB.tile([128, 2, QL], F32, tag="B")
        for mc in range(2):
            for cc in range(8):
                nc.tensor.matmul(repr_ps[:, mc, :],
                                 lhsT=wrt_t[:, cc, mc * 128:(mc + 1) * 128],
                                 rhs=capt_t[:, cc, :],
                                 start=(cc == 0), stop=(cc == 7))
        repr_sb = small.tile([128, 2, QL], BF16)
        for mc in range(2):
            nc.vector.tensor_scalar_add(repr_sb[:, mc, :], repr_ps[:, mc, :],
                                        bred_t[:, mc:mc + 1])

        L_ps = [psy.tile([128, 8, QL], F32, tag="y", name="L0"),
                psy.tile([128, 8, QL], F32, tag="y", name="L1"),
                psA.tile([128, 8, QL], F32, tag="A", name="L2")]
        for kk in range(K):
            for mc in range(8):
                nc.tensor.matmul(L_ps[kk][:, mc, :],
                                 lhsT=wpp_t[:, 0, kk, mc * 128:(mc + 1) * 128],
                                 rhs=repr_sb[:, 0, :], start=True, stop=False)
                nc.tensor.matmul(L_ps[kk][:, mc, :],
                                 lhsT=wpp_t[:, 1, kk, mc * 128:(mc + 1) * 128],
                                 rhs=repr_sb[:, 1, :], start=False, stop=True)

        # softmax over the K taps (no max-sub: |logits| ~ N(0,1))
        e_k = [small.tile([128, 8, QL], F32, name=f"ek{i}") for i in range(K)]
        for kk in range(K):
            for mc in range(8):
                nc.scalar.activation(e_k[kk][:, mc, :], L_ps[kk][:, mc, :],
                                     AF.Exp, bias=bpp_t[:, mc, kk:kk + 1])
        ssum = small.tile([128, 8, QL], F32)
        nc.vector.tensor_add(ssum, e_k[0], e_k[1])
        nc.vector.tensor_add(ssum, ssum, e_k[2])
        rinv = small.tile([128, 8, QL], F32)
        nc.vector.reciprocal(rinv, ssum)
        w_t = [const.tile([128, 8, QL], F32, name=f"w{i}") for i in range(K)]
        for kk in range(K):
            nc.vector.tensor_mul(w_t[kk], e_k[kk], rinv)

        # ---- main loop over local captions ----
        for q in range(QL):
            capb_t = cbp.tile([B, D], BF16, tag="capb")
            nc.sync.dma_start(out=capb_t, in_=capb_d.ap()[q])

            # depthwise conv build, per c-chunk with small scratch tiles
            x8 = xc8[q % 2]
            xb = xcb[q % 2]
            for cc in range(8):
                xin = x38_t[:, cc, :, :]
                mA = bld.tile([128, NB], BF16, tag="mA")
                mB = bld.tile([128, NB], BF16, tag="mB")
                mC = bld.tile([128, NB], BF16, tag="mC")
                mAo = mA.rearrange("p (b r) -> p b r", r=R)
                mBo = mB.rearrange("p (b r) -> p b r", r=R)
                mCo = mC.rearrange("p (b r) -> p b r", r=R)
                nc.vector.tensor_scalar_mul(mAo, xin[:, :, 0:36],
                                            w_t[0][:, cc, q:q + 1])
                nc.vector.tensor_scalar_mul(mBo, xin[:, :, 2:38],
                                            w_t[2][:, cc, q:q + 1])
                # Prelu(alpha=1) == identity; stays in the natural_log_exp set
                nc.scalar.activation(mCo, xin[:, :, 1:37], AF.Prelu,
                                     scale=w_t[1][:, cc, q:q + 1], alpha=1.0)
                nc.vector.tensor_add(mA, mA, mB)
                if cc < NC8:
                    nc.gpsimd.tensor_add(x8[:, cc, :], mA, mC)
                else:
                    nc.vector.tensor_add(xb[:, cc - NC8, :], mA, mC)

            A_ps = psA.tile([B, D], F32, tag="A")
            B_ps = psB.tile([B, D], F32, tag="B")

            for j in range(NCH):
                ncols = 128 if j < NCH - 1 else 64
                n0 = j * 128
                y_ps = psy.tile([128, D], F32, tag="y")
                for cp in range(NC8 // 2):
                    lhsT = x8[:, 2 * cp:2 * cp + 2, n0:n0 + ncols]
                    for h in range(2):
                        nc.tensor.matmul(
                            y_ps[:ncols, h * 512:(h + 1) * 512],
                            lhsT=lhsT,
                            rhs=wc8_t[:, 2 * cp:2 * cp + 2,
                                      h * 512:(h + 1) * 512],
                            start=(cp == 0), stop=False,
                            perf_mode=PM.DoubleRow)
                for cb in range(NCB):
                    lhsT = xb[:, cb, n0:n0 + ncols]
                    for h in range(2):
                        nc.tensor.matmul(
                            y_ps[:ncols, h * 512:(h + 1) * 512],
                            lhsT=lhsT,
                            rhs=wcb_t[:, cb, h * 512:(h + 1) * 512],
                            start=False, stop=(cb == NCB - 1))
                e_t = ep.tile([128, D], BF16, tag="e")
                p_t = ep.tile([128, D], BF16, tag="p")
                nc.scalar.activation(e_t[:ncols, :], y_ps[:ncols, :],
                                     AF.Exp, scale=1.0 / WSC)
                nc.vector.scalar_tensor_tensor(
                    p_t[:ncols, :], y_ps[:ncols, :], 1.0 / (WSC * PSC),
                    e_t[:ncols, :], OP.mult, OP.mult)
                selj = sel_t[:, j, :]
                for h in range(2):
                    hs = slice(h * 512, (h + 1) * 512)
                    nc.tensor.matmul(A_ps[:, hs], lhsT=selj, rhs=e_t[:, hs],
                                     start=(j == 0), stop=(j == NCH - 1))
                    nc.tensor.matmul(B_ps[:, hs], lhsT=selj, rhs=p_t[:, hs],
                                     start=(j == 0), stop=(j == NCH - 1))

            # epilogue: u = B/A; sims pieces accumulate into [B, QL] tiles
            lnA = qv.tile([B, D], F32, tag="lnA")
            nc.scalar.activation(lnA, A_ps, AF.Ln, scale=1.0 / PSC)
            rA = qv.tile([B, D], F32, tag="rA")
            nc.scalar.activation(rA, lnA, AF.Exp, scale=-1.0)
            u = qv.tile([B, D], F32, tag="u")
            nc.vector.tensor_mul(u, B_ps, rA)
            scr = qv.tile([B, D], F32, tag="scr")
            scr2 = qv.tile([B, D], F32, tag="scr2")
            sq = qv.tile([B, D], F32, tag="sq")
            nc.vector.scalar_tensor_tensor(scr, u, 1.0, capb_t,
                                           OP.mult, OP.mult,
                                           accum_out=dot_t[:, q:q + 1])
            nc.scalar.activation(sq, u, AF.Square,
                                 accum_out=s2u_t[:, q:q + 1])
            nc.vector.scalar_tensor_tensor(scr2, u, 1.0, bcb_t,
                                           OP.mult, OP.mult,
                                           accum_out=s2ub_t[:, q:q + 1])

        # final combine: sims = (dot + c1) / sqrt(s2u + 2 s2ub + c2)
        dotf = small.tile([B, QL], F32)
        nc.vector.tensor_add(dotf, dot_t, cst_t[:, 0:QL])
        den = small.tile([B, QL], F32)
        nc.vector.scalar_tensor_tensor(den, s2ub_t, 2.0, s2u_t,
                                       OP.mult, OP.add)
        nc.vector.tensor_scalar_add(den, den, cst_t[:, QL:QL + 1])
        lg = small.tile([B, QL], F32)
        nc.scalar.activation(lg, den, AF.Ln)
        rs = small.tile([B, QL], F32)
        nc.scalar.activation(rs, lg, AF.Exp, scale=-0.5)
        out_sb = small.tile([B, QL], F32)
        nc.vector.tensor_mul(out_sb, dotf, rs)
        nc.sync.dma_start(out=out_d.ap(), in_=out_sb)

    nc.compile()
    return nc


def _chunked(a):
    """(D, ...) -> (128, 8, ...) with d = c*128 + p."""
    return np.ascontiguousarray(
        a.reshape(8, 128, *a.shape[1:]).transpose(1, 0, *range(2, a.ndim + 1)))


def kernel(img_embed, cap_embed, lens, Wred, bred, Wproj, bproj, Wconv,
           bconv, **_unused):
    global LAST_EXEC_NS
    img_embed = np.asarray(img_embed, np.float32)
    cap0 = np.asarray(cap_embed, np.float32)[:, 0, :]        # (Q, D)
    Wred = np.asarray(Wred, np.float32)
    bred_a = np.asarray(bred, np.float32)
    Wproj = np.asarray(Wproj, np.float32)
    bproj_a = np.asarray(bproj, np.float32)
    Wconv = np.asarray(Wconv, np.float32)
    bconv_a = np.asarray(bconv, np.float32)

    if "nc" not in _CACHE:
        _CACHE["nc"] = _build_nc()
    nc = _CACHE["nc"]

    # shared host prep
    xt = np.ascontiguousarray(img_embed.transpose(2, 0, 1))   # (D, B, R)
    x38 = np.zeros((D, B, 38), np.float32)
    x38[:, :, 1:37] = xt
    x38 = _chunked(x38).astype(BF)                            # (128,8,B,38)
    wct = _chunked(np.ascontiguousarray(Wconv.T)) * WSC       # (128,8,D)
    wc8 = np.clip(wct[:, 0:NC8], -240.0, 240.0).astype(F8NP)
    wcb = wct[:, NC8:8].astype(BF)
    wrt = _chunked(np.ascontiguousarray(Wred.T)).astype(BF)
    wpp = np.ascontiguousarray(
        Wproj.reshape(D, K, DQ).transpose(2, 1, 0)
        .reshape(2, 128, K, D).transpose(1, 0, 2, 3)).astype(BF)
    sel = np.zeros((128, NCH, B), np.float32)
    n = np.arange(NB)
    sel[n % 128, n // 128, n // R] = 1.0
    selb = sel.astype(BF)
    bred_s = np.ascontiguousarray(bred_a.reshape(2, 128).T)
    bpp = _chunked(bproj_a.reshape(D, K))                     # (128,8,K)
    bcb = np.ascontiguousarray(
        np.broadcast_to(bconv_a, (B, D))).astype(np.float32)

    chat = cap0 / np.linalg.norm(cap0, axis=1, keepdims=True)  # (Q, D)
    c1 = chat @ bconv_a                                        # (Q,)
    c2 = float(bconv_a @ bconv_a)

    in_maps = []
    for c in range(N_CORES):
        qs = slice(c * QL, (c + 1) * QL)
        capq = cap0[qs]                                        # (QL, D)
        capt = _chunked(np.ascontiguousarray(capq.T)).astype(BF)
        capb = np.ascontiguousarray(
            np.broadcast_to(chat[qs][:, None, :], (QL, B, D))).astype(BF)
        cst = np.empty((B, QL + 1), np.float32)
        cst[:, 0:QL] = c1[qs][None, :]
        cst[:, QL] = c2
        in_maps.append({
            "x38": x38, "wc8": wc8, "wcb": wcb, "wrt": wrt, "wpp": wpp,
            "bred": bred_s, "bpp": bpp, "sel": selb, "bcb": bcb,
            "capt": capt, "capb": capb, "cst": cst,
        })

    trace = bool(int(os.environ.get("KTRACE", "0")))
    tdir = os.environ.get("KTRACE_DIR") or None
    res = run_bass_kernel_spmd(nc, in_maps, core_ids=list(range(N_CORES)),
                               trace=trace, tmpdir=tdir)
    LAST_EXEC_NS = res.exec_time_ns
    return np.concatenate([res.results[c]["out"] for c in range(N_CORES)],
                          axis=1)


# revision 11
# speedup vs baseline: 1.1987x; 1.1638x over previous
"""Trainium2 Bass kernel for nn_KernelProjectionT2I.

Sharding: data-parallel over the caption axis (B_cap=48 -> 6 captions per
core on 8 cores). Each core holds the full image batch + conv weights and
computes the (B_img, 6) similarity columns for its captions; the host
concatenates the per-core columns.

Math (per caption q):
  cap0 = cap_embed[q, 0]                          (1024,)
  cap_repr = Wred @ cap0 + bred                   (256,)
  wdyn = softmax_K((Wproj @ cap_repr + bproj).reshape(1024, 3))
  conv[c, n] = w0[c] x[c, r-1] + w1[c] x[c, r] + w2[c] x[c, r+1]
  y = Wconv @ conv               (bconv folded out: softmax over regions is
                                  shift-invariant; pooled = B/A + bconv)
  A = sum_r exp(y), B = sum_r y exp(y)            (gated pool, per (b, d))
  u = B/A ; sims[b, q] = <u + bconv, c_hat> / |u + bconv|   (c_hat host-normed)

Layout: n = (b, r) on PSUM partitions for y; region sums are TensorEngine
0/1-selector matmuls. Contraction c is split 6/8 chunks fp8e4 + DoubleRow
(2x PE throughput; Wconv pre-scaled x32 for fp8 range) and 2/8 chunks
bf16 (caps the fp8 quantization noise: rel err ~1.5e-2 vs the 2e-2 gate).
Depthwise build in bf16 via DVE tensor_scalar (4x) + tensor_tensor (2x);
GpSimd fuses the last tap with the bf16->fp8 cast. e/p and the selector
matmuls stay bf16. ScalarE uses only {Exp, Ln, Prelu} so one activation
table set serves the whole kernel (Prelu alpha=1 == identity for the
per-channel tap-1 multiply).
"""

import os
import numpy as np
from contextlib import ExitStack

import ml_dtypes

import concourse.bass as bass
import concourse.tile as tile
from concourse import bacc, mybir
from concourse.bass_utils import run_bass_kernel_spmd

F32 = mybir.dt.float32
BF16 = mybir.dt.bfloat16
F8 = mybir.dt.float8e4
AF = mybir.ActivationFunctionType
OP = mybir.AluOpType
PM = mybir.MatmulPerfMode

N_CORES = 8
B, R, D = 48, 36, 1024
Q = 48
QL = Q // N_CORES          # 6 captions per core
DQ, K = 256, 3
NB = B * R                 # 1728
NCH = 14                   # ceil(1728/128); chunk 13 has 64 cols
NC8 = 4                    # c-chunks 0..3 via fp8 DoubleRow
NCB = 4                    # c-chunks 4..7 via bf16

WSC = 32.0                 # Wconv pre-scale (fp8 range); exp un-scales
PSC = 4.0                  # p = (y/PSC) e^y keeps p in fp8/bf16 range

LAST_EXEC_NS = None
_CACHE = {}

BF = ml_dtypes.bfloat16
F8NP = mybir.dt.np(F8)


def _build_nc():
    nc = bacc.Bacc(trn_type="TRN2", target_bir_lowering=False,
                   num_devices=N_CORES)
    # MLP inputs (loaded first; gate the caption MLP)
    capt_d = nc.dram_tensor("capt", [128, 8, QL], BF16, kind="ExternalInput")
    wrt_d = nc.dram_tensor("wrt", [128, 8, DQ], BF16, kind="ExternalInput")
    wpp_d = nc.dram_tensor("wpp", [128, 2, K, D], BF16, kind="ExternalInput")
    bred_d = nc.dram_tensor("bred", [128, 2], F32, kind="ExternalInput")
    bpp_d = nc.dram_tensor("bpp", [128, 8, K], F32, kind="ExternalInput")
    # main loop inputs
    x38_d = nc.dram_tensor("x38", [128, 8, B, 38], BF16, kind="ExternalInput")
    x0c_d = nc.dram_tensor("x0c", [128, 8, NB], BF16, kind="ExternalInput")
    x2c_d = nc.dram_tensor("x2c", [128, 8, NB], BF16, kind="ExternalInput")
    wc8_d = nc.dram_tensor("wc8", [128, NC8, D], F8, kind="ExternalInput")
    wcb_d = nc.dram_tensor("wcb", [128, NCB, D], BF16, kind="ExternalInput")
    sel_d = nc.dram_tensor("sel", [128, NCH, B], BF16, kind="ExternalInput")
    bcb_d = nc.dram_tensor("bcb", [B, D], BF16, kind="ExternalInput")
    capb_d = nc.dram_tensor("capb", [QL, B, D], BF16, kind="ExternalInput")
    cst_d = nc.dram_tensor("cst", [B, QL + 1], F32, kind="ExternalInput")
    out_d = nc.dram_tensor("out", [B, QL], F32, kind="ExternalOutput")

    with ExitStack() as ctx:
        tc = ctx.enter_context(tile.TileContext(nc))
        const = ctx.enter_context(tc.tile_pool(name="const", bufs=1))
        bld = ctx.enter_context(tc.tile_pool(name="bld", bufs=2))
        xcp = ctx.enter_context(tc.tile_pool(name="xcp", bufs=2))
        ep = ctx.enter_context(tc.tile_pool(name="ep", bufs=2))
        qv = ctx.enter_context(tc.tile_pool(name="qv", bufs=1))
        cbp = ctx.enter_context(tc.tile_pool(name="cbp", bufs=2))
        small = ctx.enter_context(tc.tile_pool(name="small", bufs=2))
        psy = ctx.enter_context(tc.tile_pool(name="psy", bufs=2, space="PSUM"))
        psA = ctx.enter_context(tc.tile_pool(name="psA", bufs=1, space="PSUM"))
        psB = ctx.enter_context(tc.tile_pool(name="psB", bufs=1, space="PSUM"))

        atl = mybir.InstLoadActFuncSet(
            name=nc.get_next_instruction_name(), ins=[], outs=[],
            act_func_set_id=6)  # natural_log_exp_and_others: exp/ln/prelu/sq
        nc.scalar.add_instruction(atl)

        # ---- resident inputs (MLP deps first: DMA order follows issue order)
        capt_t = const.tile([128, 8, QL], BF16)
        nc.sync.dma_start(out=capt_t, in_=capt_d.ap())
        wrt_t = xcp.tile([128, 8, DQ], BF16, tag="xb", name="wrt")
        nc.sync.dma_start(out=wrt_t, in_=wrt_d.ap())
        wpp_t = xcp.tile([128, 2, K, D], BF16, tag="xb", name="wpp")
        nc.sync.dma_start(out=wpp_t, in_=wpp_d.ap())
        bred_t = const.tile([128, 2], F32)
        nc.sync.dma_start(out=bred_t, in_=bred_d.ap())
        bpp_t = const.tile([128, 8, K], F32)
        nc.sync.dma_start(out=bpp_t, in_=bpp_d.ap())
        x38_t = const.tile([128, 8, B, 38], BF16)
        nc.sync.dma_start(out=x38_t, in_=x38_d.ap())
        x0c_t = const.tile([128, 8, NB], BF16)
        nc.sync.dma_start(out=x0c_t, in_=x0c_d.ap())
        x2c_t = const.tile([128, 8, NB], BF16)
        nc.sync.dma_start(out=x2c_t, in_=x2c_d.ap())
        wc8_t = const.tile([128, NC8, D], F8)
        nc.sync.dma_start(out=wc8_t, in_=wc8_d.ap())
        wcb_t = const.tile([128, NCB, D], BF16)
        nc.sync.dma_start(out=wcb_t, in_=wcb_d.ap())
        sel_t = const.tile([128, NCH, B], BF16)
        nc.sync.dma_start(out=sel_t, in_=sel_d.ap())
        bcb_t = const.tile([B, D], BF16)
        nc.sync.dma_start(out=bcb_t, in_=bcb_d.ap())
        cst_t = const.tile([B, QL + 1], F32)
        nc.sync.dma_start(out=cst_t, in_=cst_d.ap())

        xc8 = [xcp.tile([128, NC8, NB], F8, name=f"xc8_{i}", tag="x8")
               for i in range(2)]
        xcb = [xcp.tile([128, NCB, NB], BF16, name=f"xcb_{i}", tag="xb")
               for i in range(2)]

        dot_t = const.tile([B, QL], F32)
        s2u_t = const.tile([B, QL], F32)
        s2ub_t = const.tile([B, QL], F32)

        # ---- caption MLP for all local captions ----
        repr_ps = psB.tile([128, 2, QL], F32, tag="B")
        for mc in range(2):
            for cc in range(8):
                nc.tensor.matmul(repr_ps[:, mc, :],
                                 lhsT=wrt_t[:, cc, mc * 128:(mc + 1) * 128],
                                 rhs=capt_t[:, cc, :],
                                 start=(cc == 0), stop=(cc == 7))
        repr_sb = small.tile([128, 2, QL], BF16)
        for mc in range(2):
            nc.vector.tensor_scalar_add(repr_sb[:, mc, :], repr_ps[:, mc, :],
                                        bred_t[:, mc:mc + 1])

        L_ps = [psy.tile([128, 8, QL], F32, tag="y", name="L0"),
                psy.tile([128, 8, QL], F32, tag="y", name="L1"),
                psA.tile([128, 8, QL], F32, tag="A", name="L2")]
        for kk in range(K):
            for mc in range(8):
                nc.tensor.matmul(L_ps[kk][:, mc, :],
                                 lhsT=wpp_t[:, 0, kk, mc * 128:(mc + 1) * 128],
                                 rhs=repr_sb[:, 0, :], start=True, stop=False)
                nc.tensor.matmul(L_ps[kk][:, mc, :],
                                 lhsT=wpp_t[:, 1, kk, mc * 128:(mc + 1) * 128],
                                 rhs=repr_sb[:, 1, :], start=False, stop=True)

        # softmax over the K taps (no max-sub: |logits| ~ N(0,1))
        e_k = [small.tile([128, 8, QL], F32, name=f"ek{i}") for i in range(K)]
        for kk in range(K):
            for mc in range(8):
                nc.scalar.activation(e_k[kk][:, mc, :], L_ps[kk][:, mc, :],
                                     AF.Exp, bias=bpp_t[:, mc, kk:kk + 1])
        ssum = small.tile([128, 8, QL], F32)
        nc.vector.tensor_add(ssum, e_k[0], e_k[1])
        nc.vector.tensor_add(ssum, ssum, e_k[2])
        rinv = small.tile([128, 8, QL], F32)
        nc.vector.reciprocal(rinv, ssum)
        w_t = [const.tile([128, 8, QL], F32, name=f"w{i}") for i in range(K)]
        for kk in range(K):
            nc.vector.tensor_mul(w_t[kk], e_k[kk], rinv)

        # ---- main loop over local captions ----
        for q in range(QL):
            capb_t = cbp.tile([B, D], BF16, tag="capb")
            nc.sync.dma_start(out=capb_t, in_=capb_d.ap()[q])

            # depthwise conv build, per c-chunk with small scratch tiles
            x8 = xc8[q % 2]
            xb = xcb[q % 2]
            for cc in range(8):
                mA = bld.tile([128, NB], BF16, tag="mA")
                mB = bld.tile([128, NB], BF16, tag="mB")
                mC = bld.tile([128, NB], BF16, tag="mC")
                mCo = mC.rearrange("p (b r) -> p b r", r=R)
                nc.vector.tensor_scalar_mul(mA, x0c_t[:, cc, :],
                                            w_t[0][:, cc, q:q + 1])
                nc.vector.tensor_scalar_mul(mB, x2c_t[:, cc, :],
                                            w_t[2][:, cc, q:q + 1])
                # Prelu(alpha=1) == identity; stays in the natural_log_exp set
                nc.scalar.activation(mCo, x38_t[:, cc, :, 1:37], AF.Prelu,
                                     scale=w_t[1][:, cc, q:q + 1], alpha=1.0)
                nc.vector.tensor_add(mA, mA, mB)
                if cc < NC8:
                    nc.gpsimd.tensor_add(x8[:, cc, :], mA, mC)
                else:
                    nc.vector.tensor_add(xb[:, cc - NC8, :], mA, mC)

            A_ps = psA.tile([B, D], F32, tag="A")
            B_ps = psB.tile([B, D], F32, tag="B")

            for j in range(NCH):
                ncols = 128 if j < NCH - 1 else 64
                n0 = j * 128
                y_ps = psy.tile([128, D], F32, tag="y")
                for cp in range(NC8 // 2):
                    lhsT = x8[:, 2 * cp:2 * cp + 2, n0:n0 + ncols]
                    for h in range(2):
                        nc.tensor.matmul(
                            y_ps[:ncols, h * 512:(h + 1) * 512],
                            lhsT=lhsT,
                            rhs=wc8_t[:, 2 * cp:2 * cp + 2,
                                      h * 512:(h + 1) * 512],
                            start=(cp == 0), stop=False,
                            perf_mode=PM.DoubleRow)
                for cb in range(NCB):
                    lhsT = xb[:, cb, n0:n0 + ncols]
                    for h in range(2):
                        nc.tensor.matmul(
                            y_ps[:ncols, h * 512:(h + 1) * 512],
                            lhsT=lhsT,
                            rhs=wcb_t[:, cb, h * 512:(h + 1) * 512],
                            start=False, stop=(cb == NCB - 1))
                e_t = ep.tile([128, D], BF16, tag="e")
                p_t = ep.tile([128, D], BF16, tag="p")
                nc.scalar.activation(e_t[:ncols, :], y_ps[:ncols, :],
                                     AF.Exp, scale=1.0 / WSC)
                nc.vector.scalar_tensor_tensor(
                    p_t[:ncols, :], y_ps[:ncols, :], 1.0 / (WSC * PSC),
                    e_t[:ncols, :], OP.mult, OP.mult)
                selj = sel_t[:, j, :]
                for h in range(2):
                    hs = slice(h * 512, (h + 1) * 512)
                    nc.tensor.matmul(A_ps[:, hs], lhsT=selj, rhs=e_t[:, hs],
                                     start=(j == 0), stop=(j == NCH - 1))
                    nc.tensor.matmul(B_ps[:, hs], lhsT=selj, rhs=p_t[:, hs],
                                     start=(j == 0), stop=(j == NCH - 1))

            # epilogue: u = B/A; sims pieces accumulate into [B, QL] tiles
            lnA = qv.tile([B, D], F32, tag="lnA")
            nc.scalar.activation(lnA, A_ps, AF.Ln, scale=1.0 / PSC)
            rA = qv.tile([B, D], F32, tag="rA")
            nc.scalar.activation(rA, lnA, AF.Exp, scale=-1.0)
            u = qv.tile([B, D], F32, tag="u")
            nc.vector.tensor_mul(u, B_ps, rA)
            scr = qv.tile([B, D], F32, tag="scr")
            nc.vector.scalar_tensor_tensor(scr, u, 1.0, capb_t,
                                           OP.mult, OP.mult,
                                           accum_out=dot_t[:, q:q + 1])
            nc.scalar.activation(lnA, u, AF.Square,
                                 accum_out=s2u_t[:, q:q + 1])
            nc.vector.scalar_tensor_tensor(scr, u, 1.0, bcb_t,
                                           OP.mult, OP.mult,
                                           accum_out=s2ub_t[:, q:q + 1])

        # final combine: sims = (dot + c1) / sqrt(s2u + 2 s2ub + c2)
        dotf = small.tile([B, QL], F32)
        nc.vector.tensor_add(dotf, dot_t, cst_t[:, 0:QL])
        den = small.tile([B, QL], F32)
        nc.vector.scalar_tensor_tensor(den, s2ub_t, 2.0, s2u_t,
                                       OP.mult, OP.add)
        nc.vector.tensor_scalar_add(den, den, cst_t[:, QL:QL + 1])
        lg = small.tile([B, QL], F32)
        nc.scalar.activation(lg, den, AF.Ln)
        rs = small.tile([B, QL], F32)
        nc.scalar.activation(rs, lg, AF.Exp, scale=-0.5)
        out_sb = small.tile([B, QL], F32)
        nc.vector.tensor_mul(out_sb, dotf, rs)
        nc.sync.dma_start(out=out_d.ap(), in_=out_sb)

    nc.compile()
    return nc


def _chunked(a):
    """(D, ...) -> (128, 8, ...) with d = c*128 + p."""
    return np.ascontiguousarray(
        a.reshape(8, 128, *a.shape[1:]).transpose(1, 0, *range(2, a.ndim + 1)))


def kernel(img_embed, cap_embed, lens, Wred, bred, Wproj, bproj, Wconv,
           bconv, **_unused):
    global LAST_EXEC_NS
    img_embed = np.asarray(img_embed, np.float32)
    cap0 = np.asarray(cap_embed, np.float32)[:, 0, :]        # (Q, D)
    Wred = np.asarray(Wred, np.float32)
    bred_a = np.asarray(bred, np.float32)
    Wproj = np.asarray(Wproj, np.float32)
    bproj_a = np.asarray(bproj, np.float32)
    Wconv = np.asarray(Wconv, np.float32)
    bconv_a = np.asarray(bconv, np.float32)

    if "nc" not in _CACHE:
        _CACHE["nc"] = _build_nc()
    nc = _CACHE["nc"]

    # shared host prep
    xt = np.ascontiguousarray(img_embed.transpose(2, 0, 1))   # (D, B, R)
    x38 = np.zeros((D, B, 38), np.float32)
    x38[:, :, 1:37] = xt
    x0c = np.ascontiguousarray(
        _chunked(x38[:, :, 0:36]).reshape(128, 8, NB)).astype(BF)
    x2c = np.ascontiguousarray(
        _chunked(x38[:, :, 2:38]).reshape(128, 8, NB)).astype(BF)
    x38 = _chunked(x38).astype(BF)                            # (128,8,B,38)
    wct = _chunked(np.ascontiguousarray(Wconv.T)) * WSC       # (128,8,D)
    wc8 = np.clip(wct[:, 0:NC8], -240.0, 240.0).astype(F8NP)
    wcb = wct[:, NC8:8].astype(BF)
    wrt = _chunked(np.ascontiguousarray(Wred.T)).astype(BF)
    wpp = np.ascontiguousarray(
        Wproj.reshape(D, K, DQ).transpose(2, 1, 0)
        .reshape(2, 128, K, D).transpose(1, 0, 2, 3)).astype(BF)
    sel = np.zeros((128, NCH, B), np.float32)
    n = np.arange(NB)
    sel[n % 128, n // 128, n // R] = 1.0
    selb = sel.astype(BF)
    bred_s = np.ascontiguousarray(bred_a.reshape(2, 128).T)
    bpp = _chunked(bproj_a.reshape(D, K))                     # (128,8,K)
    bcb = np.ascontiguousarray(np.broadcast_to(bconv_a, (B, D))).astype(BF)

    chat = cap0 / np.linalg.norm(cap0, axis=1, keepdims=True)  # (Q, D)
    c1 = chat @ bconv_a                                        # (Q,)
    c2 = float(bconv_a @ bconv_a)

    in_maps = []
    for c in range(N_CORES):
        qs = slice(c * QL, (c + 1) * QL)
        capq = cap0[qs]                                        # (QL, D)
        capt = _chunked(np.ascontiguousarray(capq.T)).astype(BF)
        capb = np.ascontiguousarray(
            np.broadcast_to(chat[qs][:, None, :], (QL, B, D))).astype(BF)
        cst = np.empty((B, QL + 1), np.float32)
        cst[:, 0:QL] = c1[qs][None, :]
        cst[:, QL] = c2
        in_maps.append({
            "x38": x38, "x0c": x0c, "x2c": x2c,
            "wc8": wc8, "wcb": wcb, "wrt": wrt, "wpp": wpp,
            "bred": bred_s, "bpp": bpp, "sel": selb, "bcb": bcb,
            "capt": capt, "capb": capb, "cst": cst,
        })

    trace = bool(int(os.environ.get("KTRACE", "0")))
    tdir = os.environ.get("KTRACE_DIR") or None
    res = run_bass_kernel_spmd(nc, in_maps, core_ids=list(range(N_CORES)),
                               trace=trace, tmpdir=tdir)
    LAST_EXEC_NS = res.exec_time_ns
    return np.concatenate([res.results[c]["out"] for c in range(N_CORES)],
                          axis=1)
